# revision 21
# baseline (speedup 1.0000x reference)
"""BitLinear (ternary absmean-quantized linear) on 8 TRN2 NeuronCores.

Reference math (fp32):
    gamma = mean(|W|)
    Wq    = round(clip(W / (gamma + 1e-5), -1, 1))   # ternary {-1, 0, 1}
    out   = einsum('bsi,oi->bso', x, Wq)             # x @ Wq.T

Sharding: data-parallel over tokens. x [4,2048,4096] -> 8192 tokens, each
core owns 1024 of them and computes its full [1024, 4096] output slab with
no output collective. Every core needs the full quantized W; gamma (a global
scalar) is computed cooperatively: each core abs-sums 1/8 of W (512 of the
4096 output rows), a tiny [128,1] AllReduce combines the partials, and each
core then quantizes the full W on the fly while the TensorEngine consumes it.

Ternary quantization is exact in bf16, so the matmul runs in bf16
(x rounded to bf16, Wq in {-1,0,1} exactly) with fp32 PSUM accumulation.

Device kernel layout (per core):
    xT  [4096, 1024] bf16  - this core's x slab, transposed (K-major)
    WT  [4096, 4096] f32   - full W, transposed (in_features major), replicated
    Wg  [4096,  512] f32   - this core's gamma shard (= 512 columns of WT)
    out [1024, 4096] f32

Main loop: 8 N-chunks of 512 output features. Per chunk: stream 32 K-slabs
of WT, quantize each (|w| > t indicator on DVE, sign on ACT, product on DVE)
into a resident bf16 [128, 32, 512] chunk, then 8 m-tiles x 32 k-tiles of
128x128x512 bf16 matmuls accumulating in PSUM.
"""

import numpy as np
import ml_dtypes

NCORES = 8

# Full-problem dims (hardcoded per the harness contract).
B, S, D_IN, D_OUT = 4, 2048, 4096, 4096
M_TOTAL = B * S            # 8192 tokens
M_CORE = M_TOTAL // NCORES  # 1024 tokens per core

_COMPILED = None   # cached (nc, meta)
LAST_RESULTS = None  # BassKernelResults of the most recent run (for test.py)


def build_module(m_core=M_CORE, k=D_IN, n=D_OUT, ncores=NCORES, repeat=1,
                 use_collective=True, n_collectives=1):
    """Build + compile the SPMD Bass module. Parametrized so a shrunken
    config can be validated in CoreSim. repeat>1 unrolls the whole kernel
    body multiple times inside one NEFF (for steady-state timing)."""
    import concourse.bass as bass  # noqa: F401
    import concourse.mybir as mybir
    import concourse.tile as tile
    from concourse import bacc
    from concourse import bass_isa

    f32 = mybir.dt.float32
    bf16 = mybir.dt.bfloat16
    KT = k // 128            # k-tiles of 128
    MT = m_core // 128       # m-tiles of 128
    NCHUNK = 512             # output-feature chunk width
    NCHUNKS = n // NCHUNK
    NG = n // 8              # gamma shard width (columns of WT, 8-way shard)
    G_CHUNK = min(4, KT)     # k-tiles per gamma reduce chunk
    G_CHUNKS = KT // G_CHUNK
    N_ELEMS = float(k * n)

    nc = bacc.Bacc("TRN2", target_bir_lowering=False, debug=False,
                   num_devices=ncores)
    xT = nc.dram_tensor("xT", [k, m_core], bf16, kind="ExternalInput")
    WT = nc.dram_tensor("WT", [k, n], f32, kind="ExternalInput")
    Wg = nc.dram_tensor("Wg", [k, NG], f32, kind="ExternalInput")
    out = nc.dram_tensor("out", [m_core, n], f32, kind="ExternalOutput")

    ts = bass.ts

    with tile.TileContext(nc) as tc:
        with (
            tc.tile_pool(name="xpool", bufs=1) as xpool,
            tc.tile_pool(name="gpool", bufs=2) as gpool,
            tc.tile_pool(name="wqpool", bufs=6) as wqpool,
            tc.tile_pool(name="wpool", bufs=16) as wpool,
            tc.tile_pool(name="spool", bufs=6) as spool,
            tc.tile_pool(name="opool", bufs=6) as opool,
            tc.tile_pool(name="small", bufs=2) as small,
            tc.tile_pool(name="pmain", bufs=8, space="PSUM") as pmain,
            tc.tile_pool(name="dram", bufs=2, space="DRAM") as dram,
        ):
          for _rep in range(repeat):
            # ---- gamma: local abs-sum over this core's shard ----
            acc = small.tile([128, G_CHUNKS], f32)
            for j in range(G_CHUNKS):
                gsl = gpool.tile([128, G_CHUNK, NG], f32, tag="gsl")
                src = Wg[j * G_CHUNK * 128:(j + 1) * G_CHUNK * 128, :]
                nc.sync.dma_start(gsl[:], src.rearrange("(t p) c -> p t c", p=128))
                nc.vector.tensor_reduce(
                    acc[:, j:j + 1], gsl[:], axis=mybir.AxisListType.XY,
                    op=mybir.AluOpType.add, apply_absolute_value=True)
            gpart = small.tile([128, 1], f32)
            nc.vector.tensor_reduce(
                gpart[:], acc[:], axis=mybir.AxisListType.X,
                op=mybir.AluOpType.add)

            # ---- tiny AllReduce of per-partition partials ----
            gsum = small.tile([128, 1], f32)
            if ncores > 1 and use_collective:
                cin = dram.tile([128, 1], f32)
                nc.sync.dma_start(cin[:], gpart[:])
                for ci in range(n_collectives):
                    cout = dram.tile([128, 1], f32, tag=f"cout{ci}",
                                     name=f"cout{ci}")
                    nc.gpsimd.collective_compute(
                        "AllReduce", mybir.AluOpType.add,
                        replica_groups=[list(range(ncores))],
                        ins=[cin[:].opt()], outs=[cout[:].opt()])
                    cin = cout
                nc.sync.dma_start(gsum[:], cout[:])
            else:
                # timing/TimelineSim variant: no collective (gamma from the
                # local shard only -- numerically wrong, timing-equivalent)
                nc.vector.tensor_copy(gsum[:], gpart[:])

            # sum across partitions, result broadcast to all partitions
            gtot = small.tile([128, 1], f32)
            nc.gpsimd.partition_all_reduce(
                gtot[:], gsum[:], channels=128, reduce_op=bass_isa.ReduceOp.add)

            # threshold t = 0.5 * (gamma + 1e-5)
            # Wq = (w > t) - (w < -t)  in {-1, 0, 1}
            tsb = small.tile([128, 1], f32)
            nc.vector.tensor_scalar(
                tsb[:], gtot[:], 0.5 / N_ELEMS, 0.5e-5,
                mybir.AluOpType.mult, mybir.AluOpType.add)
            ntsb = small.tile([128, 1], f32)
            nc.vector.tensor_scalar(
                ntsb[:], tsb[:], -1.0, None, mybir.AluOpType.mult)

            # ---- resident xT: [128, KT, m_core] bf16 ----
            # Loaded lazily: slab kt's DMA is interleaved into chunk 0's
            # W stream (emitted just before W slab kt) so the first matmul
            # only waits for slab 0, not the whole 8.4 MB.
            xsb = xpool.tile([128, KT, m_core], bf16)
            xr = xT[:, :].rearrange("(t p) m -> p t m", p=128)

            # ---- main loop over output-feature chunks ----
            # kt-outer / mt-inner: each quantized W slab feeds the MT
            # parallel PSUM accumulation groups (one bank per m-tile)
            # immediately, so the PE ramps up right after the first slab is
            # quantized and each slab dies young (small wq pool).
            for c in range(NCHUNKS):
                ps = [pmain.tile([128, NCHUNK], f32, tag="ps", name=f"ps{mt}")
                      for mt in range(MT)]
                for kt in range(KT):
                    if c == 0:
                        nc.sync.dma_start(xsb[:, kt, :], xr[:, kt, :])
                    wtmp = wpool.tile([128, NCHUNK], f32, tag="wtmp")
                    nc.sync.dma_start(
                        wtmp[:], WT[ts(kt, 128), ts(c, NCHUNK)])
                    neg = spool.tile([128, NCHUNK], bf16, tag="neg")
                    nc.vector.tensor_scalar(
                        neg[:], wtmp[:], ntsb[:], None, mybir.AluOpType.is_lt)
                    wqt = wqpool.tile([128, NCHUNK], bf16, tag="wq")
                    nc.vector.scalar_tensor_tensor(
                        wqt[:], wtmp[:], tsb[:], neg[:],
                        mybir.AluOpType.is_gt, mybir.AluOpType.subtract)
                    for mt in range(MT):
                        nc.tensor.matmul(
                            ps[mt][:], xsb[:, kt, ts(mt, 128)], wqt[:],
                            start=(kt == 0), stop=(kt == KT - 1))
                for mt in range(MT):
                    osb = opool.tile([128, NCHUNK], f32, tag="osb")
                    nc.vector.tensor_copy(osb[:], ps[mt][:])
                    nc.sync.dma_start(out[ts(mt, 128), ts(c, NCHUNK)], osb[:])

    nc.compile()
    meta = dict(m_core=m_core, k=k, n=n, ncores=ncores, NG=NG)
    return nc, meta


def _get_compiled():
    global _COMPILED
    if _COMPILED is None:
        _COMPILED = build_module()
    return _COMPILED


def make_in_maps(x, W, m_core=M_CORE, ncores=NCORES):
    """Host-side shard prep. x [B,S,D_IN] f32, W [D_OUT,D_IN] f32."""
    k = W.shape[1]
    n = W.shape[0]
    ng = n // ncores
    x2 = np.asarray(x, dtype=np.float32).reshape(-1, k)
    xb = x2.astype(ml_dtypes.bfloat16)
    WT = np.ascontiguousarray(np.asarray(W, dtype=np.float32).T)  # [k, n]
    in_maps = []
    for c in range(ncores):
        xTc = np.ascontiguousarray(xb[c * m_core:(c + 1) * m_core, :].T)
        Wgc = np.ascontiguousarray(WT[:, c * ng:(c + 1) * ng])
        in_maps.append({"xT": xTc, "WT": WT, "Wg": Wgc})
    return in_maps


def kernel(input, W):
    """Full inputs in, full output out. Shards internally across 8 cores."""
    global LAST_RESULTS
    from concourse import bass_utils

    nc, meta = _get_compiled()
    in_maps = make_in_maps(input, W)
    res = bass_utils.run_bass_kernel_spmd(
        nc, in_maps, core_ids=list(range(NCORES)))
    LAST_RESULTS = res
    out = np.concatenate([res.results[c]["out"] for c in range(NCORES)], axis=0)
    return out.reshape(B, S, D_OUT).astype(np.float32)
